# revision 16
# baseline (speedup 1.0000x reference)
"""GCN forward on 8 Trainium2 NeuronCores.

Reference computation:
  h1 = relu(GCNConv(x, edge_index; w_conv, b_conv))      [20000, 32]
  h3 = relu(h1.flatten() @ w_fc1.T + b_fc1)              [128]
  out = relu(h3 @ w_fc2.T + b_fc2)                       [1, 20000]

Strategy (all 8 cores, SPMD, one NEFF):
  - GCNConv aggregation as a DENSE matmul with the A-matrix as the MOVING
    operand: A_hat = D^-1/2 (A+I) D^-1/2; (A+I) holds small integer edge
    counts, exact in fp8e4m3. dinv[src] folds into x rows on host,
    dinv[dst] applied post-matmul. Stationary = H'' tile [128 src, 32 feat],
    moving = A tile [128 src, 500 dst] fp8. The 160 src tiles round-robin
    over FOUR 128x32 PE column tiles (tile_position=(0,32j)) -> 4 matmuls
    stream concurrently on disjoint sub-arrays (HW-measured 3.8x: 66 vs
    251 ns/matmul). Each col-tile accumulates its 40 src tiles into its
    own 32-partition group of one [128, 500] PSUM bank.
  - The 4 partial sums are reduced AND transposed in one matmul per
    125-dst block: lhsT = S[128, 125] (psum copied to SBUF bf16),
    rhs = M4 = [I32;I32;I32;I32] [128, 32] -> out[125, 32] =
    sum_g4 S[32 g4 + f, dl]: h1^T directly (node partition, feat free).
    2-op epilogue in transposed layout via relu(x*d + b) = relu(x + b/d)*d:
    DVE add of bconv/dinv (a const table), then one ACT relu with
    dinv[dst] as a per-partition scale -> h1T bf16.
  - H'' exchange in TILE-TRANSPOSED layout: each core ships [128, 20*32]
    (tile-major, per-partition contiguous), AllGather -> [1024, 640]; the
    SBUF reload is 8 long-contiguous DMAs.
  - A packed per-partition-contiguous [128, 160*2500] fp8 in dst-chunk-
    major order (5 chunks of 500 dst), streamed in 40-tile (2.56 MB)
    sub-chunks, triple buffered (HW: pure A stream runs at 426 GB/s;
    deeper prefetch measured slower — it delays the hp reload at the
    head). Post-col-tiling the kernel is HBM-bound on this stream.
  - fc1 column-sharded; g-columns ordered dst-block-major: g = jk*32 + f
    pairs with h1T column (jk, f), jk = node//125, contraction = 125
    nodes (pad rows of w1pack zeroed, h1T pad partitions memset). w1
    stored as fp8-e3m4 * 1024 (10.5 MB/core in 5 x 2MB buffers; e2e rel
    err ~1.4e-2, under the 2e-2 gate). fc1 keeps w1 STATIONARY (128-col
    fp8 FWL loads + 1-col matmuls, HW-measured 35 ns/g -- the w1-moving
    form measured 61 ns/g); 128-g chunk right after each dst chunk's
    reduce, accumulating psum [128, 1]. AllReduce the partials.
  - fc2 column-blocked: 20 matmuls lhsT = w2-block [128, 128] bf16,
    rhs = h3 [128, 1] -> psum [128, 20]; 128-lane epilogue; host
    un-permutes the [128, 20] block layout.
"""
import numpy as np
import ml_dtypes

N = 20000
IN_FEAT = 128
CF = 32            # conv out feats
FC1 = 128
NC_ = 8            # cores
NS = N // NC_      # 2500 nodes per core
LT = 20            # local src tiles per core (last partial: 68 rows)
TT = NC_ * LT      # 160 total src tiles
JW = 5             # dst chunks of 500
JC = NS // JW      # 500 dst cols per chunk
CT = 4             # PE column tiles (agg round-robin)
KC = 125           # dst sub-block (reduce-transpose granularity)
KB = JC // KC      # 4 reduce blocks per chunk
AU = 40            # A src tiles per DMA sub-chunk (2.56 MB)
ABUFS = 3          # A sub-chunk buffers
NG = 640           # fc1 contraction columns (20 blocks * 32 f)
GMAX = 128         # g-columns per w1 DMA buffer / fc1 chunk (2 MB)
W1BUFS = 3         # fc1 weight chunk buffers

_BF16 = ml_dtypes.bfloat16
_F8 = ml_dtypes.float8_e4m3
_F8E3 = ml_dtypes.float8_e3m4
W1SCALE = 1024.0   # fc1 weights stored as e3m4 * W1SCALE (rel err ~1%)


def _host_prep(x, edge_index, w_conv, b_conv, w_fc1, b_fc1, w_fc2, b_fc2):
    src = edge_index[0].astype(np.int64)
    dst = edge_index[1].astype(np.int64)
    deg = np.bincount(dst, minlength=N).astype(np.float32) + 1.0
    dinv = (1.0 / np.sqrt(deg)).astype(np.float32)

    x = np.asarray(x, np.float32)
    w_conv = np.asarray(w_conv, np.float32)
    b_conv = np.asarray(b_conv, np.float32)
    w_fc1 = np.asarray(w_fc1, np.float32)
    b_fc1 = np.asarray(b_fc1, np.float32)
    w_fc2 = np.asarray(w_fc2, np.float32)
    b_fc2 = np.asarray(b_fc2, np.float32)

    lut = np.arange(16).astype(_F8)  # exact small-int -> fp8e4m3
    wconvb = w_conv.astype(_BF16)
    bfc1c = np.ascontiguousarray(b_fc1.reshape(FC1, 1))
    # M4 = stacked 32x32 identities: reduce-transpose matmul rhs
    m4 = np.zeros((IN_FEAT, CF), _BF16)
    for g4 in range(CT):
        m4[g4 * CF:(g4 + 1) * CF] = np.eye(CF, dtype=np.float32)
    xs = dinv[:, None] * x  # fold dinv[src] into x rows

    # dinvT[p, jk] = dinv[base + 125*jk + p]; bcd[p, jk*32+f] =
    # b_conv[f] / dinvT[p, jk]  (relu(x*d + b) = relu(x + b/d)*d, d > 0)
    # -- per core below.

    # local-tile mapping for a global src id s: tile 20*(s//2500) +
    # (s%2500)//128, partition (s%2500)%128
    s_tile = 20 * (src // NS) + (src % NS) // 128
    s_part = (src % NS) % 128

    in_maps = []
    for c in range(NC_):
        base = c * NS
        # xt: [128 feat, 2560 nodes] bf16 zero-padded, dinv pre-folded
        xt = np.zeros((IN_FEAT, LT * 128), _BF16)
        xt[:, :NS] = xs[base:base + NS].T.astype(_BF16)
        dv = np.zeros((IN_FEAT, JW * KB), np.float32)
        dv[:KC] = dinv[base:base + NS].reshape(JW * KB, KC).T
        dvt = np.ascontiguousarray(dv)
        bcd = np.zeros((IN_FEAT, JW * KB * CF), np.float32)
        bcd[:KC] = (b_conv[None, None, :]
                    / dv[:KC, :, None]).reshape(KC, JW * KB * CF)
        bcd = np.ascontiguousarray(bcd)
        # A_pack[p, (j*160 + i)*500 + dl] = count(src-tile(i,p) -> dst
        # base + 500j + dl): dst-chunk-major so aggregation completes one
        # 500-col output chunk at a time (lets fc1 overlap agg)
        cnt = np.zeros((128, TT * NS), np.uint8)
        m = (dst >= base) & (dst < base + NS)
        dl = dst[m] - base
        np.add.at(cnt, (s_part[m],
                        ((dl // JC) * TT + s_tile[m]) * JC + dl % JC), 1)
        v = np.arange(base, base + NS)
        vl = v - base
        np.add.at(cnt, ((v % NS) % 128,
                        ((vl // JC) * TT + 20 * c + (v % NS) // 128) * JC + vl % JC), 1)
        assert cnt.max() < 16, cnt.max()
        apack = lut[cnt]
        del cnt
        # w1pack[p, g*128 + o] = w_fc1[o, 32*(base + 125*jk + p) + f]
        # for g = jk*32 + f, p < 125; pad rows (125:128) zero
        w1c = w_fc1[:, base * CF:(base + NS) * CF]          # [128, 80000]
        w1r = w1c.reshape(FC1, JW * KB, KC, CF)             # [o, jk, p, f]
        w1p = np.zeros((128, JW * KB, CF, FC1), np.float32)  # [p, jk, f, o]
        w1p[:KC] = w1r.transpose(2, 1, 3, 0)
        w1p = np.ascontiguousarray(
            w1p.reshape(128, NG * FC1) * W1SCALE).astype(_F8E3)
        # w2pack[p, b*128 + q] = w_fc2[base + 128b + q, p]
        w2pad = np.zeros((LT * 128, FC1), np.float32)
        w2pad[:NS] = w_fc2[base:base + NS]
        w2p = w2pad.reshape(LT, 128, FC1).transpose(2, 0, 1)
        w2p = np.ascontiguousarray(w2p.reshape(FC1, LT * 128)).astype(_BF16)
        # bias2[q, b] = b_fc2[base + 128b + q] (zero pad)
        b2 = np.zeros(LT * 128, np.float32)
        b2[:NS] = b_fc2[base:base + NS]
        b2 = np.ascontiguousarray(b2.reshape(LT, 128).T)
        in_maps.append({
            "xt": xt,
            "wconv": wconvb,
            "dinvT": dvt,
            "bcd": bcd,
            "m4": m4,
            "apack": apack,
            "w1pack": w1p,
            "bfc1": bfc1c,
            "w2pack": w2p,
            "bias2": b2,
        })
    return in_maps


def _build_bass(timing_reps=None, lite=False):
    # lite: timing-diagnostic build — emit only the first A sub-chunk and
    # first w1 buffer (keeps all other phases intact) to measure overhead.
    import concourse.bass as bass
    import concourse.mybir as mybir
    import concourse.tile as tile
    from concourse import bacc

    F32, BF16, F8 = mybir.dt.float32, mybir.dt.bfloat16, mybir.dt.float8e4
    F8E3 = mybir.dt.float8e3
    import contextlib
    nc = bacc.Bacc("TRN2", target_bir_lowering=False, debug=False,
                   num_devices=1 if timing_reps else NC_)

    xt = nc.dram_tensor("xt", [IN_FEAT, LT * 128], BF16, kind="ExternalInput")
    wconv = nc.dram_tensor("wconv", [IN_FEAT, CF], BF16, kind="ExternalInput")
    dinvT = nc.dram_tensor("dinvT", [IN_FEAT, JW * KB], F32, kind="ExternalInput")
    bcd = nc.dram_tensor("bcd", [IN_FEAT, JW * KB * CF], F32, kind="ExternalInput")
    m4 = nc.dram_tensor("m4", [IN_FEAT, CF], BF16, kind="ExternalInput")
    apack = nc.dram_tensor("apack", [128, TT * NS], F8, kind="ExternalInput")
    w1pack = nc.dram_tensor("w1pack", [128, NG * FC1], F8E3, kind="ExternalInput")
    bfc1 = nc.dram_tensor("bfc1", [FC1, 1], F32, kind="ExternalInput")
    w2pack = nc.dram_tensor("w2pack", [FC1, LT * 128], BF16, kind="ExternalInput")
    bias2 = nc.dram_tensor("bias2", [128, LT], F32, kind="ExternalInput")
    out = nc.dram_tensor("out", [128, LT], F32, kind="ExternalOutput")

    hq_in = nc.dram_tensor("hq_in", [128, LT * CF], BF16)
    hq_out = nc.dram_tensor("hq_out", [NC_ * 128, LT * CF], BF16,
                            addr_space="Shared")
    p_in = nc.dram_tensor("p_in", [FC1, 1], F32)
    p_out = nc.dram_tensor("p_out", [FC1, 1], F32, addr_space="Shared")

    with tile.TileContext(nc) as tc:
        with tc.tile_pool(name="const", bufs=1) as cp, \
             tc.tile_pool(name="abuf", bufs=ABUFS) as apool, \
             tc.tile_pool(name="w1buf", bufs=W1BUFS) as wpool, \
             tc.tile_pool(name="work", bufs=2) as wp, \
             tc.tile_pool(name="psA", bufs=2, space="PSUM") as ppA, \
             tc.tile_pool(name="psB", bufs=2, space="PSUM") as ppB, \
             tc.tile_pool(name="psC", bufs=1, space="PSUM") as ppC:

            xt_sb = cp.tile([IN_FEAT, LT * 128], BF16, tag="xt")
            nc.sync.dma_start(out=xt_sb[:], in_=xt[:])
            wconv_sb = cp.tile([IN_FEAT, CF], BF16, tag="wconv")
            nc.sync.dma_start(out=wconv_sb[:], in_=wconv[:])
            dinvT_sb = cp.tile([IN_FEAT, JW * KB], F32, tag="dinvT")
            nc.sync.dma_start(out=dinvT_sb[:], in_=dinvT[:])
            bcd_sb = cp.tile([IN_FEAT, JW * KB * CF], F32, tag="bcd")
            nc.sync.dma_start(out=bcd_sb[:], in_=bcd[:])
            m4_sb = cp.tile([IN_FEAT, CF], BF16, tag="m4")
            nc.sync.dma_start(out=m4_sb[:], in_=m4[:])
            bfc1_sb = cp.tile([FC1, 1], F32, tag="bfc1")
            nc.sync.dma_start(out=bfc1_sb[:], in_=bfc1[:])
            w2_sb = cp.tile([FC1, LT * 128], BF16, tag="w2")
            nc.sync.dma_start(out=w2_sb[:], in_=w2pack[:])
            bias2_sb = cp.tile([128, LT], F32, tag="bias2")
            nc.sync.dma_start(out=bias2_sb[:], in_=bias2[:])

            hp = cp.tile([128, TT * CF], BF16, tag="hp")

            loop_cm = tc.For_i(0, timing_reps, 1) if timing_reps else contextlib.nullcontext()
            loop_cm.__enter__()

            # ---- S1: H''_tileT = (dinv*x) @ w_conv, [128, 20*32] bf16 ----
            # (pad rows of xt are zero -> pad rows of H'' are zero)
            hq_sb = cp.tile([128, LT * CF], BF16, tag="hq")
            for k in range(LT):
                ps = ppA.tile([128, CF], F32, space="PSUM", tag="mm")
                nc.tensor.matmul(out=ps[:], lhsT=xt_sb[:, k * 128:(k + 1) * 128],
                                 rhs=wconv_sb[:], start=True, stop=True)
                nc.vector.tensor_copy(out=hq_sb[:, k * CF:(k + 1) * CF], in_=ps[:])
            nc.sync.dma_start(out=hq_in[:], in_=hq_sb[:])

            # ---- S2: AllGather H'' (tile-transposed blocks) ----
            if timing_reps:
                nc.sync.dma_start(out=hq_out[:128], in_=hq_in[:])
            else:
                nc.gpsimd.collective_compute(
                    "AllGather", mybir.AluOpType.bypass,
                    replica_groups=[list(range(NC_))],
                    ins=[hq_in[:]], outs=[hq_out[:]])

            # ---- prefetch: first A sub-chunks run during the AllGather ----
            def a_chunk_dma(j, u):
                ab = apool.tile([128, AU * JC], F8, tag="apk", name="apk")
                off = (j * TT + u * AU) * JC
                nc.sync.dma_start(out=ab[:], in_=apack[:, off:off + AU * JC])
                return ab

            def w1_chunk_dma(t):
                wb = wpool.tile([128, GMAX * FC1], F8E3, tag="w1", name="wb")
                nc.sync.dma_start(out=wb[:],
                                  in_=w1pack[:, t * GMAX * FC1:(t + 1) * GMAX * FC1])
                return wb

            abufs = {(0, u): a_chunk_dma(0, u) for u in range(1 if lite else ABUFS)}
            wbufs = {0: w1_chunk_dma(0)}
            if not lite:
                wbufs[1] = w1_chunk_dma(1)

            # ---- S3: load gathered H'' into SBUF [128, 160*32] ----
            for c in range(NC_):
                nc.sync.dma_start(out=hp[:, c * LT * CF:(c + 1) * LT * CF],
                                  in_=hq_out[c * 128:(c + 1) * 128, :])

            # ---- S4-S7 fused: dst-chunk-outer aggregation (4-way PE
            # column tiling) with reduce-transpose, epilogue and fc1
            # chunks interleaved into the A-stream's DMA-wait gaps ----
            h1T = cp.tile([128, NG], BF16, tag="h1T")
            nc.vector.memset(h1T[:], 0.0)  # pad partitions 125:128 -> 0
            psf = ppC.tile([128, 24], F32, space="PSUM", tag="fc")

            def s5_chunk(j, psj):
                # psj [128, 500] = 4 col-tile partials; for each 125-dst
                # block: reduce+transpose matmul, then 2-op epilogue
                sc = wp.tile([128, JC], BF16, tag="sc", name="sc")
                nc.vector.tensor_copy(out=sc[:], in_=psj[:])
                for k in range(KB):
                    ps2 = ppA.tile([128, CF], F32, space="PSUM", tag="mm",
                                   name="ps2")
                    nc.tensor.matmul(out=ps2[:KC, :],
                                     lhsT=sc[:, k * KC:(k + 1) * KC],
                                     rhs=m4_sb[:], start=True, stop=True)
                    c = j * KB + k
                    u1 = wp.tile([128, CF], F32, tag="ep1", name="u1")
                    nc.vector.tensor_tensor(out=u1[:KC, :], in0=ps2[:KC, :],
                                            in1=bcd_sb[:KC, c * CF:(c + 1) * CF],
                                            op=mybir.AluOpType.add)
                    nc.scalar.activation(out=h1T[:KC, c * CF:(c + 1) * CF],
                                         in_=u1[:KC, :],
                                         func=mybir.ActivationFunctionType.Relu,
                                         scale=dinvT_sb[:KC, c:c + 1])

            def fc1_chunk(t, g_stop):
                wb = wbufs.pop(t)
                for gl in range(GMAX):
                    g = t * GMAX + gl
                    nc.tensor.matmul(out=psf[:, 0:1],
                                     lhsT=wb[:, gl * FC1:(gl + 1) * FC1],
                                     rhs=h1T[:, g:g + 1],
                                     start=(g == 0), stop=(g == g_stop))

            g_stop = GMAX - 1 if lite else NG - 1
            for j in range(1 if lite else JW):
                psj = ppB.tile([128, JC], F32, space="PSUM", tag="agg",
                               name="psj")
                for u in range(1 if lite else TT // AU):
                    ab = abufs.pop((j, u)) if (j, u) in abufs \
                        else a_chunk_dma(j, u)
                    if not lite and (j, u) in ((0, 2), (1, 2), (2, 2)):
                        wbufs[j + 2] = w1_chunk_dma(j + 2)
                    for il in range(AU):
                        i = u * AU + il
                        jj = i % CT
                        nc.tensor.matmul(
                            out=psj[32 * jj:32 * (jj + 1), :],
                            lhsT=hp[:, i * CF:(i + 1) * CF],
                            rhs=ab[:, il * JC:(il + 1) * JC],
                            tile_position=(0, 32 * jj),
                            start=(i < CT),
                            stop=(i >= (AU if lite else TT) - CT))
                s5_chunk(j, psj)
                fc1_chunk(j, g_stop)
            p_sb = cp.tile([FC1, 1], F32, tag="p_sb")
            nc.vector.tensor_copy(out=p_sb[:], in_=psf[:, 0:1])
            nc.sync.dma_start(out=p_in[:], in_=p_sb[:])

            # ---- S8: AllReduce partials, h3 = relu(sum + b_fc1) bf16 ----
            if timing_reps:
                nc.sync.dma_start(out=p_out[:], in_=p_in[:])
            else:
                nc.gpsimd.collective_compute(
                    "AllReduce", mybir.AluOpType.add,
                    replica_groups=[list(range(NC_))],
                    ins=[p_in[:]], outs=[p_out[:]])
            h3f = cp.tile([FC1, 1], F32, tag="h3f")
            nc.sync.dma_start(out=h3f[:], in_=p_out[:])
            h3 = cp.tile([FC1, 1], BF16, tag="h3")
            nc.scalar.activation(out=h3[:], in_=h3f[:],
                                 func=mybir.ActivationFunctionType.Relu,
                                 bias=bfc1_sb[:], scale=1.0 / W1SCALE)

            # ---- S9: fc2 blocks: psum[128, 20]; epilogue on 128 lanes ----
            for b in range(LT):
                nc.tensor.matmul(out=psf[:, 4 + b:5 + b],
                                 lhsT=w2_sb[:, b * 128:(b + 1) * 128],
                                 rhs=h3[:], start=(b == 0), stop=(b == LT - 1))
            o_sb = cp.tile([128, LT], F32, tag="o_sb")
            nc.vector.tensor_tensor(out=o_sb[:], in0=psf[:, 4:4 + LT],
                                    in1=bias2_sb[:], op=mybir.AluOpType.add)
            nc.scalar.activation(out=o_sb[:], in_=o_sb[:],
                                 func=mybir.ActivationFunctionType.Relu)
            nc.sync.dma_start(out=out[:], in_=o_sb[:])
            loop_cm.__exit__(None, None, None) if timing_reps else None

    nc.finalize()
    return nc


_CACHED = {}


def kernel(**inputs) -> np.ndarray:
    from concourse.bass_utils import run_bass_kernel_spmd

    in_maps = _host_prep(**inputs)
    if "nc" not in _CACHED:
        _CACHED["nc"] = _build_bass()
    nc = _CACHED["nc"]
    res = run_bass_kernel_spmd(nc, in_maps, core_ids=list(range(NC_)))
    # out[p, b] = q-value for node base + 128*b + p
    outs = []
    for c in range(NC_):
        o = np.asarray(res.results[c]["out"])  # [128, 20]
        outs.append(o.T.reshape(-1)[:NS])
    return np.concatenate(outs).reshape(1, N)
